# revision 11
# baseline (speedup 1.0000x reference)
"""Trainium2 Bass kernel for nn_Adaptive_dilatedConv (dense_cnn), v2.

Reference computation (per image):
  logits = einsum('bchw,kc->bkhw', x, attn_w) + attn_b        # [B,3,H,W]
  attn   = softmax(logits, axis=1)
  convs_k = depthwise3x3(x, dw_w[k], dilation d_k) + dw_b[k]  # [B,C,H,W] x3
  fused  = sum_k convs_k * (attn_k + 1)
  out    = einsum('bchw,oc->bohw', fused, out_w) + out_b

Distribution: data parallel over batch (16 images over 8 cores).

Per-core schedule (all engines balanced, two images software-pipelined):
  - x arrives pre-padded from the host as [2, 128, 74*74] bf16 frames, plus
    fp8(e4m3) copies (and fp8 residual copies) of the same frames.
  - attention: per-hw-tile matmuls (lhsT = padded-frame interior views) give
    transposed logits [hw, (j,k)]; softmax in that layout with exp(attn_b)
    folded as host immediates; TensorE transpose; k-major DRAM bounce gives
    (1 + attn_k) as three [128, 4096] partition-broadcast bf16 tensors
    ((1+attn_1) pre-scaled by 1/64 to undo the fp8 weight scaling below).
  - branch d=2 runs ENTIRELY on the TensorEngine in fp8 DoubleRow mode:
    host precomputes g[c,t,o] = out_w[o,c] * dw_w[1][c,t] * 64, split into
    e4m3 main + e5m2 residual; per 4-row output block one PSUM chain
    accumulates 9 taps x (main, w-resid[, x-resid]) DoubleRow matmuls over
    contiguous 296-element runs of the padded frame (74-col geometry keeps
    every tap view one contiguous run; pad columns are computed and
    discarded).  This fuses conv1 AND its 1x1 out-conv at 1/4 the bf16
    matmul cost; the conv bias rides the gw combine as a per-partition
    scalar.
  - branches d=1 and d=5 are elementwise, spread over three engines:
    ScalarE activation-Copy quarter-products with DVE adds, and DVE
    half-products with gpsimd tensor_tensor adds (gpsimd lacks the fused
    scalar_tensor_tensor opcode on hardware).
  - (1 + attn_k) applied in place on DVE; out conv P_a = OW @ m_k
    accumulates both weighted branches in PSUM (4 matmuls per 256-col
    block), interleaved into the NEXT image's fp8 chain window so the
    TensorEngine never drains.
  - combines split by hardware rules (GPSIMD may not access PSUM):
    two fp8 chains share one 2-bank PSUM tile and ACT/DVE drain them in a
    single paired op gw = P_g/64 + q1, gpsimd applies (1+attn_1) in SBUF,
    DVE fuses osb = (P_a + out_b) + gw into f32 staging eighths;
    contiguous DMAs store the output.  The second image
    skips the fp8 x-residual matmuls (~1% output error, 2x under the
    tolerance) which unblocks its chain window and shortens the tail.
  - emission is explicitly interleaved (chains / taps / combines / next
    image's attention) because each engine executes its stream in order.
"""

import sys
from contextlib import ExitStack

import numpy as np

sys.path.insert(0, "/opt/trn_rl_repo")

import concourse.bass as bass  # noqa: E402
import concourse.bacc as bacc  # noqa: E402
import concourse.mybir as mybir  # noqa: E402
import concourse.tile as tile  # noqa: E402
from concourse.masks import make_identity  # noqa: E402

F32 = mybir.dt.float32
BF16 = mybir.dt.bfloat16
FP8 = mybir.dt.float8e4
FP8E5 = mybir.dt.float8e5

N_CORES = 8
B, C, H, W = 16, 256, 64, 64
PB = B // N_CORES
PAD = 5
WP = W + 2 * PAD  # 74
FW = WP * WP  # 5476
HW = H * W  # 4096
GSCALE = 64.0
GINV = 1.0 / GSCALE
NBLK = 16  # 4-row blocks per ok-chunk
BR = 4  # rows per block
RUN = BR * WP  # 296-element contiguous run per DoubleRow rhs half

# taps (of branch d=2) that get the fp8 x-residual correction matmul
XCOMP_TAPS = (0, 1, 2, 3, 4, 5, 6, 7, 8)

AluOp = mybir.AluOpType
ActFn = mybir.ActivationFunctionType
PerfMode = mybir.MatmulPerfMode

DILS = {0: 1, 1: 5}  # elementwise branch index -> dilation


def build_bass(u_vals, reps=1):
    nc = bacc.Bacc()

    xpe_d = nc.declare_dram_parameter("xpe", [PB, 2, 128, FW], BF16,
                                      isOutput=False)
    x8_d = nc.declare_dram_parameter("x8", [PB, 128, 2, FW], FP8,
                                     isOutput=False)
    xr8_d = nc.declare_dram_parameter("xr8", [PB, 128, 2, FW], FP8,
                                      isOutput=False)
    g8_d = nc.declare_dram_parameter("g8", [128, 2, 9, 2, 128], FP8,
                                     isOutput=False)
    gr8_d = nc.declare_dram_parameter("gr8", [128, 2, 9, 2, 128], FP8E5,
                                      isOutput=False)
    q1_d = nc.declare_dram_parameter("q1s", [128, 2], F32, isOutput=False)
    ob_d = nc.declare_dram_parameter("obs", [128, 2], F32, isOutput=False)
    owt_d = nc.declare_dram_parameter("owt", [128, 2, 256], BF16,
                                      isOutput=False)
    awp_d = nc.declare_dram_parameter("awp", [128, 2, 3], BF16, isOutput=False)
    dwp_d = nc.declare_dram_parameter("dwp", [128, 2, 2, 10], F32,
                                      isOutput=False)
    out_d = nc.declare_dram_parameter("out", [PB, C, H, W], F32, isOutput=True)

    with tile.TileContext(nc) as tc:
        _body(nc, tc, xpe_d, x8_d, xr8_d, g8_d, gr8_d, q1_d, ob_d, owt_d,
              awp_d, dwp_d, out_d, u_vals, reps)
    nc.finalize()
    return nc


def _body(nc, tc, xpe_d, x8_d, xr8_d, g8_d, gr8_d, q1_d, ob_d, owt_d, awp_d,
          dwp_d, out_d, u_vals, reps):
    ctx = ExitStack()
    with ctx:
        singles = ctx.enter_context(tc.tile_pool(name="singles", bufs=1))
        xpep = ctx.enter_context(tc.tile_pool(name="xpep", bufs=2))
        x8p = ctx.enter_context(tc.tile_pool(name="x8p", bufs=2))
        xr8p = ctx.enter_context(tc.tile_pool(name="xr8p", bufs=1))
        a1p = ctx.enter_context(tc.tile_pool(name="a1p", bufs=1))
        accp = ctx.enter_context(tc.tile_pool(name="accp", bufs=2))
        prods = ctx.enter_context(tc.tile_pool(name="prods", bufs=2))
        dprods = ctx.enter_context(tc.tile_pool(name="dprods", bufs=2))
        smalls = ctx.enter_context(tc.tile_pool(name="smalls", bufs=1))
        gwp = ctx.enter_context(tc.tile_pool(name="gwp", bufs=15))
        ostp = ctx.enter_context(tc.tile_pool(name="ostp", bufs=2))
        dramp = ctx.enter_context(tc.tile_pool(name="dramp", bufs=2,
                                               space="DRAM"))
        ps_l = ctx.enter_context(tc.tile_pool(name="ps_l", bufs=1,
                                              space="PSUM"))
        ps_t = ctx.enter_context(tc.tile_pool(name="ps_t", bufs=1,
                                              space="PSUM"))
        psg = ctx.enter_context(tc.tile_pool(name="psg", bufs=2,
                                             space="PSUM"))
        psa = ctx.enter_context(tc.tile_pool(name="psa", bufs=2,
                                             space="PSUM"))

        # ---- resident weights ----
        g8_sb = singles.tile([128, 2, 9, 2, 128], FP8)
        nc.gpsimd.dma_start(out=g8_sb, in_=g8_d[:, :, :, :, :])
        gr8_sb = singles.tile([128, 2, 9, 2, 128], FP8E5)
        nc.gpsimd.dma_start(out=gr8_sb, in_=gr8_d[:, :, :, :, :])
        q1_sb = singles.tile([128, 2], F32)
        nc.sync.dma_start(out=q1_sb, in_=q1_d[:, :])
        ob_sb = singles.tile([128, 2], F32)
        nc.sync.dma_start(out=ob_sb, in_=ob_d[:, :])
        owt_sb = singles.tile([128, 2, 256], BF16)
        nc.gpsimd.dma_start(out=owt_sb, in_=owt_d[:, :, :])
        awp_sb = singles.tile([128, 2, 3], BF16)
        nc.sync.dma_start(out=awp_sb, in_=awp_d[:, :, :])
        dwp_sb = singles.tile([128, 2, 2, 10], F32)
        nc.gpsimd.dma_start(out=dwp_sb, in_=dwp_d[:, :, :, :])
        ident = singles.tile([128, 128], F32)
        make_identity(nc, ident)

        def w_ap(ck, ki, t):
            return dwp_sb[:, ck, ki, t : t + 1]

        def b_ap(ck, ki):
            return dwp_sb[:, ck, ki, 9:10]

        st = [dict() for _ in range(PB)]

        # ---------------- emitters ----------------
        def emit_loads(img, defer_xr8=False, defer_x8=False):
            s = st[img]
            s["xpe"] = [None, None]
            for ck in (0, 1):
                t_ = xpep.tile([128, WP, WP], BF16, tag=f"xpe{ck}",
                               name=f"xpe{ck}")
                nc.sync.dma_start(
                    out=t_.rearrange("p a b -> p (a b)"),
                    in_=xpe_d[img, ck, :, :],
                )
                s["xpe"][ck] = t_
            if not defer_x8:
                s["x8"] = x8p.tile([128, 2, FW], FP8, tag="x8", name="x8t")
                nc.scalar.dma_start(out=s["x8"], in_=x8_d[img, :, :, :])
            if XCOMP_TAPS and not defer_xr8:
                pass
            if XCOMP_TAPS and not defer_xr8:
                s["xr8"] = xr8p.tile([128, 2, FW], FP8, tag="xr8", name="xr8t")
                nc.gpsimd.dma_start(out=s["xr8"], in_=xr8_d[img, :, :, :])

        def emit_x8_sp(img):
            s = st[img]
            s["x8"] = x8p.tile([128, 2, FW], FP8, tag="x8", name="x8t")
            nc.sync.dma_start(out=s["x8"], in_=x8_d[img, :, :, :])

        def emit_xr8_sp(img):
            # deferred second-image residual load on the SP queue: with
            # xr8p bufs=1 it blocks until the previous image's chains
            # release the tile, so it must not sit at the head of a busy
            # engine's stream
            s = st[img]
            if XCOMP_TAPS:
                s["xr8"] = xr8p.tile([128, 2, FW], FP8, tag="xr8", name="xr8t")
                nc.sync.dma_start(out=s["xr8"], in_=xr8_d[img, :, :, :])

        def emit_logits(img):
            # one padded-frame ROW per tile so the stationary AP has a
            # single free dim (walrus requirement); logits land as
            # [64(hw-part), (j,k)]
            s = st[img]
            lps = ps_l.tile([64, 192], F32, tag="lps")
            for j in range(64):
                for ck in range(2):
                    nc.tensor.matmul(
                        lps[:, 3 * j : 3 * j + 3],
                        lhsT=s["xpe"][ck][:, PAD + j, PAD : PAD + W],
                        rhs=awp_sb[:, ck, :],
                        start=(ck == 0),
                        stop=(ck == 1),
                    )
            s["lps"] = lps

        def emit_softmax(img):
            s = st[img]
            esb = smalls.tile([64, 192], F32, tag="esb")
            nc.scalar.activation(esb, s["lps"][:, :], ActFn.Exp)
            e3 = esb.rearrange("p (j k) -> p j k", k=3)
            ssum = smalls.tile([64, 64], F32, tag="ssum")
            nc.vector.tensor_scalar(
                out=ssum, in0=e3[:, :, 0], scalar1=float(u_vals[0]),
                scalar2=None, op0=AluOp.mult,
            )
            for k in (1, 2):
                nc.vector.scalar_tensor_tensor(
                    out=ssum, in0=e3[:, :, k], scalar=float(u_vals[k]),
                    in1=ssum, op0=AluOp.mult, op1=AluOp.add,
                )
            rsum = smalls.tile([64, 64], F32, tag="rsum")
            nc.vector.reciprocal_approx_fast(rsum, ssum)
            a1t = smalls.tile([64, 192], F32, tag="a1t")  # cols k*64+j
            a1t3 = a1t.rearrange("p (k j) -> p k j", j=64)
            for k in range(3):
                nc.vector.scalar_tensor_tensor(
                    out=a1t3[:, k, :], in0=e3[:, :, k],
                    scalar=float(u_vals[k]), in1=ssum,
                    op0=AluOp.mult, op1=AluOp.add,
                )
                nc.vector.tensor_tensor(
                    out=a1t3[:, k, :], in0=a1t3[:, k, :], in1=rsum,
                    op=AluOp.mult,
                )
            s["a1t"] = a1t

        def emit_transpose(img):
            # [64, 192] -> two [96, 64] transposes; rows stay (k*64 + j)
            s = st[img]
            tps = ps_t.tile([96, 2, 64], F32, tag="tps")
            for h in range(2):
                nc.tensor.transpose(tps[:, h, :],
                                    s["a1t"][:, 96 * h : 96 * h + 96],
                                    ident[0:64, 0:64])
            a1rows = smalls.tile([96, 2, 64], BF16, tag="a1rows")
            nc.vector.tensor_copy(a1rows, tps[:, :, :])
            s["a1rows"] = a1rows

        def emit_a1dma(img, ks=(1, 0, 2), write=True):
            s = st[img]
            if write:
                a1dram = dramp.tile([2, 96, 64], BF16, tag="a1dram")
                nc.sync.dma_start(
                    out=a1dram.rearrange("h r w -> r h w"), in_=s["a1rows"])
                s["a1dram"] = a1dram
                s["a1sb"] = [None, None, None]
            a1dram = s["a1dram"]
            for k in ks:
                a1k = a1p.tile([128, HW], BF16, tag=f"a1{k}")
                bsrc = bass.AP(
                    tensor=a1dram.tensor,
                    offset=a1dram.offset + k * HW,
                    ap=[[0, 128], [1, HW]],
                )
                nc.sync.dma_start(out=a1k, in_=bsrc)
                s["a1sb"][k] = a1k

        def emit_chain(img, ok, nb):
            s = st[img]
            xcomp = XCOMP_TAPS if img == 0 else ()
            if nb % 2 == 0:
                pgpair = psg.tile([128, 2, 512], F32, tag="pg", name="pgpair")
                s["pg"][(ok, nb // 2)] = pgpair
            pg = s["pg"][(ok, nb // 2)][:, nb % 2, :]
            h0 = nb * BR
            x8 = s["x8"]
            first = True
            for t in range(9):
                dy, dx = (t // 3 - 1) * 2, (t % 3 - 1) * 2
                base = (PAD + h0 + dy) * WP + PAD + dx
                nc.tensor.matmul(
                    pg[:, 0:RUN], lhsT=g8_sb[:, :, t, ok, :],
                    rhs=x8[:, :, base : base + RUN],
                    start=first, stop=False,
                    perf_mode=PerfMode.DoubleRow,
                )
                first = False
            for t in range(9):
                dy, dx = (t // 3 - 1) * 2, (t % 3 - 1) * 2
                base = (PAD + h0 + dy) * WP + PAD + dx
                nc.tensor.matmul(
                    pg[:, 0:RUN], lhsT=gr8_sb[:, :, t, ok, :],
                    rhs=x8[:, :, base : base + RUN],
                    start=False, stop=(t == 8 and not xcomp),
                    perf_mode=PerfMode.DoubleRow,
                )
            for i, t in enumerate(xcomp):
                dy, dx = (t // 3 - 1) * 2, (t % 3 - 1) * 2
                base = (PAD + h0 + dy) * WP + PAD + dx
                nc.tensor.matmul(
                    pg[:, 0:RUN], lhsT=g8_sb[:, :, t, ok, :],
                    rhs=s["xr8"][:, :, base : base + RUN],
                    start=False, stop=(i == len(xcomp) - 1),
                    perf_mode=PerfMode.DoubleRow,
                )

        def emit_gw(img, ok, nbp, eng="act"):
            # paired PSUM drain (GPSIMD may not access PSUM on hardware):
            # gw = P_g/64 + q1 for two 4-row blocks in one op; the
            # (1+attn_1) factor is applied on gpsimd at combine time
            s = st[img]
            pg = s["pg"].pop((ok, nbp))
            gw = gwp.tile([128, 2, BR * W], BF16, tag="gw")
            pgv = pg[:, :, 0:RUN].rearrange("p b (r q) -> p b r q", q=WP)[
                :, :, :, 0:W]
            if eng == "act":
                nc.scalar.activation(gw.rearrange("p b f -> p (b f)"),
                                     pgv, ActFn.Identity,
                                     bias=q1_sb[:, ok : ok + 1], scale=GINV)
            else:
                nc.vector.tensor_scalar(
                    out=gw.rearrange("p b f -> p (b f)"), in0=pgv,
                    scalar1=GINV, scalar2=q1_sb[:, ok : ok + 1],
                    op0=AluOp.mult, op1=AluOp.add,
                )
            s["gw"][(ok, nbp)] = gw

        def emit_pa(img, ok, nb):
            s = st[img]
            off = nb * BR * W
            pa = psa.tile([128, BR * W], F32, tag="pa")
            firstmm = True
            for ck in range(2):
                for ki in range(2):
                    nc.tensor.matmul(
                        pa[:, 0 : BR * W],
                        lhsT=owt_sb[:, ck, ok * 128 : ok * 128 + 128],
                        rhs=s["m"][(ki, ck)][:, off : off + BR * W],
                        start=firstmm,
                        stop=(ck == 1 and ki == 1),
                    )
                    firstmm = False
            s["pa"][(ok, nb)] = pa

        def emit_osb(img, ok, nb, eng="gp"):
            s = st[img]
            egt = nb // 2
            key = (ok, egt)
            if key not in s["ost"]:
                s["ost"][key] = ostp.tile([128, HW // 8], F32, tag="ost",
                                          name="ost")
            ost = s["ost"][key]
            loff = (nb % 2) * BR * W
            pa = s["pa"].pop((ok, nb))
            gw = s["gw"][(ok, nb // 2)]
            off = nb * BR * W
            if nb % 2 == 0:
                poff = nb * BR * W
                nc.gpsimd.tensor_tensor(
                    out=gw.rearrange("p b f -> p (b f)"),
                    in0=gw.rearrange("p b f -> p (b f)"),
                    in1=s["a1sb"][1][:, poff : poff + 2 * BR * W],
                    op=AluOp.mult,
                )
            else:
                s["gw"].pop((ok, nb // 2))
            nc.vector.scalar_tensor_tensor(
                out=ost[:, loff : loff + BR * W], in0=pa[:, 0 : BR * W],
                scalar=ob_sb[:, ok : ok + 1], in1=gw[:, nb % 2, :],
                op0=AluOp.add, op1=AluOp.add,
            )
            if nb % 2 == 1:
                ost = s["ost"].pop(key)
                # drain-phase combines (eng="dve") put their store on the
                # then-idle ACT queue; steady-state stores stay on SP
                dq = nc.sync if eng == "gp" else nc.scalar
                dq.dma_start(
                    out=out_d[img, ok * 128 : ok * 128 + 128,
                              egt * 8 : egt * 8 + 8, :],
                    in_=ost.rearrange("p (a b) -> p a b", b=W),
                )

        def emit_merge(img):
            s = st[img]
            s["m"] = {}
            for ki in range(2):
                a1k = s["a1sb"][0] if ki == 0 else s["a1sb"][2]
                e_ = nc.vector if ki == 0 else nc.gpsimd
                for ck in range(2):
                    av = s["accs"][(ki, ck)].rearrange("p h w -> p (h w)")
                    e_.tensor_tensor(out=av, in0=av, in1=a1k,
                                     op=AluOp.mult)
                    s["m"][(ki, ck)] = av


        # ---- elementwise tap op queues ----
        def tap_view(img, ck, ki, t):
            d = DILS[ki]
            dy, dx = (t // 3 - 1) * d, (t % 3 - 1) * d
            return st[img]["xpe"][ck][:, PAD + dy : PAD + dy + H,
                                      PAD + dx : PAD + dx + W]

        def make_tap_queues(img):
            s = st[img]
            s["accs"] = {}
            s["pg"] = {}
            s["gw"] = {}
            s["pa"] = {}
            s["ost"] = {}
            for ki in range(2):
                for ck in range(2):
                    s["accs"][(ki, ck)] = accp.tile(
                        [128, H, W], BF16, tag=f"acc{ki}{ck}", name=f"acc{ki}{ck}")

            def gp_first(ki, ck, t):
                def f():
                    nc.gpsimd.tensor_scalar(
                        out=s["accs"][(ki, ck)], in0=tap_view(img, ck, ki, t),
                        scalar1=w_ap(ck, ki, t), scalar2=b_ap(ck, ki),
                        op0=AluOp.mult, op1=AluOp.add,
                    )
                return f

            def gp_stt(ki, ck, t):
                def f():
                    a = s["accs"][(ki, ck)]
                    nc.gpsimd.scalar_tensor_tensor(
                        out=a, in0=tap_view(img, ck, ki, t),
                        scalar=w_ap(ck, ki, t), in1=a,
                        op0=AluOp.mult, op1=AluOp.add,
                    )
                return f

            def dve_first(ki, ck, t):
                def f():
                    nc.vector.tensor_scalar(
                        out=s["accs"][(ki, ck)], in0=tap_view(img, ck, ki, t),
                        scalar1=w_ap(ck, ki, t), scalar2=b_ap(ck, ki),
                        op0=AluOp.mult, op1=AluOp.add,
                    )
                return f

            def dve_stt(ki, ck, t):
                # fused acc += w*x in one DVE pass (1x rate); allocates no
                # product tile so it can never ring-deadlock
                def f():
                    a = s["accs"][(ki, ck)]
                    nc.vector.scalar_tensor_tensor(
                        out=a, in0=tap_view(img, ck, ki, t),
                        scalar=w_ap(ck, ki, t), in1=a,
                        op0=AluOp.mult, op1=AluOp.add,
                    )
                return f

            def tap_half(ck, ki, t, hf):
                d = DILS[ki]
                dy, dx = (t // 3 - 1) * d, (t % 3 - 1) * d
                r0 = PAD + dy + hf * (H // 2)
                return s["xpe"][ck][:, r0 : r0 + H // 2,
                                    PAD + dx : PAD + dx + W]

            def tap_q(ck, ki, t, qf):
                d = DILS[ki]
                dy, dx = (t // 3 - 1) * d, (t % 3 - 1) * d
                r0 = PAD + dy + qf * (H // 4)
                return s["xpe"][ck][:, r0 : r0 + H // 4,
                                    PAD + dx : PAD + dx + W]

            def act_prod(ki, ck, t, hf):
                # quarter-height products keep the prods pool small; only
                # ACT allocates from it (consumers are DVE adds, same order)
                def f():
                    p = prods.tile([128, H // 4, W], BF16, tag="prod")
                    nc.scalar.activation(p, tap_q(ck, ki, t, hf),
                                         ActFn.Copy, bias=0.0,
                                         scale=w_ap(ck, ki, t))
                    s["pend"][(ki, ck, t, hf)] = p
                return f

            def dve_add(ki, ck, t, qf):
                def f():
                    a = s["accs"][(ki, ck)]
                    av = a[:, qf * (H // 4) : (qf + 1) * (H // 4), :]
                    p = s["pend"].pop((ki, ck, t, qf))
                    nc.vector.tensor_tensor(out=av, in0=av, in1=p,
                                            op=AluOp.add)
                return f

            s["pend"] = {}
            s["dpend"] = {}

            def dve_hprod(ki, ck, t, hf):
                # half-height DVE product feeding a gpsimd add (gpsimd has
                # no scalar_tensor_tensor opcode on hardware)
                def f():
                    p = dprods.tile([128, H // 2, W], BF16, tag="dprod",
                                    name="dprod")
                    nc.vector.tensor_scalar(
                        out=p, in0=tap_half(ck, ki, t, hf),
                        scalar1=w_ap(ck, ki, t), scalar2=None, op0=AluOp.mult,
                    )
                    s["dpend"][(ki, ck, t, hf)] = p
                return f

            def gp_add(ki, ck, t, hf):
                def f():
                    a = s["accs"][(ki, ck)]
                    av = a[:, hf * (H // 2) : (hf + 1) * (H // 2), :]
                    p = s["dpend"].pop((ki, ck, t, hf))
                    nc.gpsimd.tensor_tensor(out=av, in0=av, in1=p,
                                            op=AluOp.add)
                return f

            # gp-anchored chains: firsts direct, then half-adds
            gp_taps = []
            for t in range(1, 9):
                gp_taps.append((0, 1, t))
                gp_taps.append((1, 1, t))
            gp_halves = [(ki, ck, t, hf) for (ki, ck, t) in gp_taps
                         for hf in (0, 1)]
            gp_q = [gp_first(0, 1, 0), gp_first(1, 1, 0)]
            gp_q += [gp_add(*key) for key in gp_halves]
            dqp = [dve_hprod(*key) for key in gp_halves]
            # act-route chains
            act_order = []
            for t in range(1, 9):
                act_order.append((0, 0, t))
                act_order.append((1, 0, t))
            act_halves = [(ki, ck, t, qf) for (ki, ck, t) in act_order
                          for qf in (0, 1, 2, 3)]
            act_q = [act_prod(*key) for key in act_halves]
            adds = [dve_add(*key) for key in act_halves]
            # DVE stream: firsts, then interleave gp-products and act-adds
            dve_q = [dve_first(0, 0, 0), dve_first(1, 0, 0)]
            pi, ai = 0, 0
            while pi < len(dqp) or ai < len(adds):
                for _ in range(2):
                    if pi < len(dqp):
                        dve_q.append(dqp[pi])
                        pi += 1
                if ai < len(adds):
                    dve_q.append(adds[ai])
                    ai += 1
            s["gp_q"] = gp_q
            s["act_q"] = act_q
            s["dve_q"] = dve_q

        def pump(q, n):
            c = 0
            while q and c < n:
                q.pop(0)()
                c += 1

        # ---------------- master schedule ----------------
        def two_images(i0, i1):
            g_order = [(ok, nb) for ok in range(2) for nb in range(NBLK)]
            emit_loads(i0)
            emit_logits(i0)
            make_tap_queues(i0)
            emit_softmax(i0)
            for ok, nb in g_order[0:3]:
                emit_chain(i0, ok, nb)
            emit_transpose(i0)
            emit_loads(i1, defer_xr8=True, defer_x8=True)
            emit_a1dma(i0, ks=(1,))
            emit_x8_sp(i1)
            emit_a1dma(i0, ks=(0, 2), write=False)
            emit_logits(i1)
            emit_softmax(i1)
            for ok, nb in g_order[3:6]:
                emit_chain(i0, ok, nb)
            emit_transpose(i1)

            # Combined chain window: both images' fp8 chains interleave
            # through the taps of BOTH images, so the TensorEngine never
            # drains while ACT/DVE are saturated.  Drains (which need no
            # attention data) are emitted ahead of each step's tap work.
            make_tap_queues(i1)
            chain_q = []
            q0 = [(i0,) + g_order[k] for k in range(6, 32)]
            q1 = [(i1,) + g_order[k] for k in range(32)]
            while q0 or q1:
                if q0:
                    chain_q.append(q0.pop(0))
                if q1:
                    chain_q.append(q1.pop(0))
            gwi = 0
            ready = [(i0, 0, 0), (i0, 0, 1), (i0, 0, 2)]
            for _ in range(2):
                emit_gw(*ready.pop(0), eng="dve")

            def chain_step(n):
                for _ in range(n):
                    if chain_q:
                        img, ok, nb = chain_q.pop(0)
                        emit_chain(img, ok, nb)
                        if nb % 2 == 1:
                            ready.append((img, ok, nb // 2))

            def drain_ready(keep=0):
                nonlocal gwi
                while len(ready) > keep:
                    emit_gw(*ready.pop(0), eng=("act", "act", "dve")[gwi % 3])
                    gwi += 1

            for step in range(17):
                chain_step(2)
                drain_ready()
                pump(st[i0]["act_q"], 4)
                pump(st[i0]["dve_q"], 6)
                pump(st[i0]["gp_q"], 2)
            pump(st[i0]["act_q"], 99)
            pump(st[i0]["dve_q"], 99)
            pump(st[i0]["gp_q"], 99)
            emit_merge(i0)
            emit_a1dma(i1)

            paq = list(g_order)
            osq = list(g_order)
            for step in range(14):
                chain_step(2)
                drain_ready()
                if step >= 3:
                    for _ in range(3):
                        if paq:
                            emit_pa(i0, *paq.pop(0))
                pump(st[i1]["act_q"], 5)
                pump(st[i1]["dve_q"], 7)
                pump(st[i1]["gp_q"], 3)
                if step >= 4:
                    for _ in range(3):
                        if len(osq) > len(paq) + 2 and osq:
                            emit_osb(i0, *osq.pop(0))
            chain_step(99)
            drain_ready()
            pump(st[i1]["act_q"], 99)
            pump(st[i1]["dve_q"], 99)
            pump(st[i1]["gp_q"], 99)
            for ok, nb in paq:
                emit_pa(i0, ok, nb)
            for ok, nb in osq:
                emit_osb(i0, ok, nb)
            emit_merge(i1)

            # P_a1 + osb1 drain
            paq1 = list(g_order)
            osq1 = list(g_order)
            di = 0
            while paq1 or osq1:
                for _ in range(2):
                    if paq1:
                        emit_pa(i1, *paq1.pop(0))
                while len(osq1) > len(paq1) and osq1:
                    emit_osb(i1, *osq1.pop(0),
                             eng="gp" if di % 2 == 0 else "dve")
                    di += 1

        for _ in range(reps):
            two_images(0, 1)


def make_in_maps(x, dw_w, dw_b, attn_w, attn_b, out_w, out_b):
    """Host-side packing. Returns (in_maps list for 8 cores, u_vals)."""
    import ml_dtypes

    bf16 = ml_dtypes.bfloat16
    fp8 = ml_dtypes.float8_e4m3
    fp8e5 = ml_dtypes.float8_e5m2

    x = np.asarray(x, np.float32)
    xpad = np.zeros((B, C, WP, WP), np.float32)
    xpad[:, :, PAD : PAD + H, PAD : PAD + W] = x
    xpe = np.ascontiguousarray(
        xpad.reshape(B, 2, 128, FW).astype(bf16)
    )  # [B, ck, c, FW]
    x8 = xpad.astype(fp8)
    xr = xpad - x8.astype(np.float32)
    xr8 = xr.astype(fp8)
    # [B, c(128), ck, FW] so per-image one DMA fills [128, 2, FW]
    x8_l = np.ascontiguousarray(
        x8.reshape(B, 2, 128, FW).transpose(0, 2, 1, 3)
    )
    xr8_l = np.ascontiguousarray(
        xr8.reshape(B, 2, 128, FW).transpose(0, 2, 1, 3)
    )

    w1 = dw_w[1].reshape(C, 9)  # [c, t]
    g = (out_w.T[:, None, :] * w1[:, :, None]) * GSCALE  # [c, t, o]
    g8 = g.astype(fp8)
    gr8 = (g - g8.astype(np.float32)).astype(fp8e5)
    # layout [c_lo(128), ck, t, ok, o_lo(128)]
    g8_l = np.ascontiguousarray(
        g8.reshape(2, 128, 9, 2, 128).transpose(1, 0, 2, 3, 4)
    )
    gr8_l = np.ascontiguousarray(
        gr8.reshape(2, 128, 9, 2, 128).transpose(1, 0, 2, 3, 4)
    )
    q1 = (out_w.astype(np.float64) @ dw_b[1].astype(np.float64)).astype(
        np.float32
    )
    q1s = np.ascontiguousarray(q1.reshape(2, 128).T)
    obs = np.ascontiguousarray(out_b.reshape(2, 128).T.astype(np.float32))
    owt = np.ascontiguousarray(
        np.stack([out_w.T[:128], out_w.T[128:]], axis=1)
    ).astype(bf16)  # [c_lo, ck, o]
    awp = np.ascontiguousarray(
        np.stack([attn_w.T[:128], attn_w.T[128:]], axis=1)
    ).astype(bf16)  # [c_lo, ck, k]
    # dwp: [c_lo, ck, ki(0:d=1,1:d=5), 10]
    dwp = np.zeros((128, 2, 2, 10), np.float32)
    for ki, kk in ((0, 0), (1, 2)):
        wk = dw_w[kk].reshape(C, 9)
        for ck in range(2):
            cs = slice(ck * 128, ck * 128 + 128)
            dwp[:, ck, ki, :9] = wk[cs]
            dwp[:, ck, ki, 9] = dw_b[kk][cs]
    u_vals = np.exp(attn_b.astype(np.float64)).astype(np.float32)

    in_maps = []
    for i in range(N_CORES):
        sl = slice(i * PB, (i + 1) * PB)
        in_maps.append({
            "xpe": np.ascontiguousarray(xpe[sl]),
            "x8": np.ascontiguousarray(x8_l[sl]),
            "xr8": np.ascontiguousarray(xr8_l[sl]),
            "g8": g8_l,
            "gr8": gr8_l,
            "q1s": q1s,
            "obs": obs,
            "owt": owt,
            "awp": awp,
            "dwp": dwp,
        })
    return in_maps, u_vals


def kernel(**inputs) -> np.ndarray:
    in_maps, u_vals = make_in_maps(
        np.asarray(inputs["x"], np.float32),
        np.asarray(inputs["dw_w"], np.float32),
        np.asarray(inputs["dw_b"], np.float32),
        np.asarray(inputs["attn_w"], np.float32),
        np.asarray(inputs["attn_b"], np.float32),
        np.asarray(inputs["out_w"], np.float32),
        np.asarray(inputs["out_b"], np.float32),
    )
    nc = build_bass(u_vals)

    from concourse.bass_utils import run_bass_kernel_spmd

    res = run_bass_kernel_spmd(nc, in_maps, core_ids=list(range(N_CORES)))
    outs = [res.results[i]["out"] for i in range(N_CORES)]
    return np.concatenate(outs, axis=0).astype(np.float32)


if __name__ == "__main__":
    nc = build_bass([1.0, 1.0, 1.0])
    print("built ok")
